# revision 19
# baseline (speedup 1.0000x reference)
"""Trainium2 Bass kernel for ConditionalCrossAttention (DAB-DETR style).

Reference computation (per batch b):
    qc = query @ Wqc.T + bqc ; qp = query_pos @ Wqp.T + bqp ; qs = qsine @ Wqs.T + bqs
    kc = key @ Wkc.T + bkc   ; kp = key_pos @ Wkp.T + bkp   ; v = value @ Wv.T + bv
    q_full = concat_heads(qc+qp, qs)   # (N, H, 64)
    k_full = concat_heads(kc+kp, kp)   # (L, H, 64)
    attn = softmax(q_full . k_full / 8) ; out = attn @ v_heads
    out = out @ Wo.T + bo ; return query + out

Sharding: data-parallel over batch B=8 across the 8 NeuronCores; each core
computes one batch element end to end (no collectives).

v3 design (all-bf16 matmuls, bias algebra folded away, drain-op minimized):
 - Softmax over l is invariant to per-n and constant logit shifts, so the
   K-side biases and bq.bk cancel entirely.  The only surviving bias term is
   h_l = bq_full . k_features(l), handled as a multiplicative row factor
   E = exp(h/8) on V (and on the denominator column), computed on the HOST
   from the inputs and shipped as a small (L, H) bf16 tensor.
 - V bias: attn weights sum to 1, so +bv passes through PV; folded host-side
   into the output projection bias bo' = bo + Wo bv.
 - Projections: interleaved [k;kp] x H "KF" (512, L) and "QF" (512, N) in
   bf16 out of PSUM; KF evictions are consolidated to 1024-element ScalarE
   copies (2 head-blocks per PSUM tile), DVE evicts V with the E row-factor
   multiply fused (broadcast in1).
 - Attention jobs = (head-pair block, parity, group of <=3 l-chunks); one
   [128,3,512] PSUM tile per job gives 900-element exp ops (HW per-op
   overhead dominates, so fewer/bigger drain ops win).  QK matmuls contract
   K=64 at base partitions 0/64 (disjoint PE row groups); PV accumulates
   into per-parity PSUM banks at partitions 0/64 (disjoint col groups).
   QK for job j+1 is emitted before exp/PV of job j so the PE never stalls
   behind the exponentials.  exp is alternated between ScalarE (LUT exp)
   and DVE (Schraudolph bit-trick exp via int16 affine + bf16 bitcast).
 - Normalization: reciprocal of the E-weighted denominator row, broadcast
   over 32 partitions on GpSimd, multiply on VectorE.
 - O-projection in bf16, residual add with fp32 query, DMA out fp32.
"""

import sys

for _p in ("/opt/trn_rl_repo",):
    if _p not in sys.path:
        sys.path.insert(0, _p)

import numpy as np

import concourse.bass as bass
import concourse.mybir as mybir
import concourse.tile as tile
from concourse import bacc
from concourse.bass_utils import run_bass_kernel_spmd

B, N, L, C, H = 8, 300, 4096, 256, 8
DH = C // H  # 32
P = 128
KC = C // P  # 2 contraction chunks of 128
LG = 8  # l groups of 512 for projection streaming
LW = L // LG  # 512
LCH = L // P  # 32 l-chunks of 128 for attention
NCHUNKS = [(0, 128), (128, 128), (256, 44)]  # n tiling of 300
GSZ = 2  # l-chunks per attention job

F32 = mybir.dt.float32
BF16 = mybir.dt.bfloat16
I16 = mybir.dt.int16
AF = mybir.ActivationFunctionType
ALU = mybir.AluOpType

SCALE = 0.125
# Schraudolph exp constants for bf16 bit pattern: bits = round(x*As + Bs)
SCH_A = SCALE * 1.4426950408889634 * 128.0
SCH_B = 16256.0 - 8.5

# engine-balance weights for the exp split (relative, HW-calibrated)
COST_SC_EXP = 1500.0
COST_DVE_EXP = 1500.0
SC_ATTN_FIXED = 0.0
DVE_ATTN_FIXED = 13.0e3  # recip + norm-mul + residual


def build_nc(reps=1, variant="full", sc_exp_cost=COST_SC_EXP, dve_exp_cost=COST_DVE_EXP,
             gp_bcast=True):
    """variant: full | allsc | alldve | noattn | noproj | noexp | nodma
    (ablations for HW timing attribution; only "full" is numerically correct).
    sc_exp_cost/dve_exp_cost: relative weights for the exp engine balance.
    gp_bcast: broadcast the reciprocal row on GpSimd instead of PE+ScalarE."""
    nc = bacc.Bacc(trn_type="TRN2", debug=False, enable_partition_id=False)

    def din(name, shape, dt=BF16):
        return nc.dram_tensor(name, list(shape), dt, kind="ExternalInput").ap()

    # transposed activations (channels, tokens), bf16
    qT = din("qT", (C, N))
    qpT = din("qpT", (C, N))
    qsT = din("qsT", (C, N))
    # group-major layout [g][p][k][x]: each DMA group is one contiguous
    # 256KB block with 2KB per-partition lines
    keyG = din("keyG", (LG, P, KC, LW))
    kposG = din("kposG", (LG, P, KC, LW))
    valG = din("valG", (LG, P, KC, LW))
    qnat = din("qnat", (N, C), F32)  # natural query for the residual
    # host-prepped weights (interleaved, unbiased)
    Ak = din("Ak", (C, 512))
    Bk = din("Bk", (C, 512))
    Aq = din("Aq", (C, 512))
    Bq = din("Bq", (C, 512))
    Cq = din("Cq", (C, 512))
    WvT = din("WvT", (C, C))
    WoT = din("WoT", (C, C))
    E1 = din("E1", (P, LCH, H))  # exp(h/8) row factors, l = c*128+p
    bo_r = din("bo_r", (1, C))  # bo + Wo bv
    out_d = nc.dram_tensor("out", [N, C], F32, kind="ExternalOutput").ap()

    # (c, x) dram tensors viewed as (partition, chunk, x)
    def pkx(ap):
        return ap.rearrange("(k p) x -> p k x", p=P)

    with tile.TileContext(nc) as tc:
        with (
            tc.tile_pool(name="const", bufs=1) as const,
            tc.tile_pool(name="persist", bufs=1) as persist,
            tc.tile_pool(name="kin", bufs=4) as kin_pool,
            tc.tile_pool(name="pt", bufs=4) as pt_pool,
            tc.tile_pool(name="fin", bufs=2) as fin_pool,
            tc.tile_pool(name="wk", bufs=3, space="PSUM") as wk,
            tc.tile_pool(name="o2p", bufs=2, space="PSUM") as o2p,
        ):
            # ---- constants / weights (one-time DMAs via SWDGE queue) ----
            Ak_sb = const.tile([P, KC, 512], BF16)
            Bk_sb = const.tile([P, KC, 512], BF16)
            Aq_sb = const.tile([P, KC, 512], BF16)
            Bq_sb = const.tile([P, KC, 512], BF16)
            Cq_sb = const.tile([P, KC, 512], BF16)
            WvT_sb = const.tile([P, KC, C], BF16)
            WoT_sb = const.tile([P, KC, C], BF16)
            E1_sb = const.tile([P, LCH, H], BF16)
            bo_sb = const.tile([1, C], BF16)
            qnat_sb = const.tile([P, 3, C], F32)
            qT_sb = persist.tile([P, KC, N], BF16)
            qpT_sb = persist.tile([P, KC, N], BF16)
            qsT_sb = persist.tile([P, KC, N], BF16)
            nc.gpsimd.dma_start(out=Ak_sb, in_=pkx(Ak))
            nc.gpsimd.dma_start(out=Bk_sb, in_=pkx(Bk))
            nc.gpsimd.dma_start(out=WvT_sb, in_=pkx(WvT))
            nc.gpsimd.dma_start(out=qT_sb, in_=pkx(qT))
            nc.gpsimd.dma_start(out=qpT_sb, in_=pkx(qpT))
            nc.gpsimd.dma_start(out=qsT_sb, in_=pkx(qsT))
            nc.gpsimd.dma_start(out=E1_sb, in_=E1)
            for dst, src in ((Aq_sb, Aq), (Bq_sb, Bq), (Cq_sb, Cq), (WoT_sb, WoT)):
                nc.gpsimd.dma_start(out=dst, in_=pkx(src))
            nc.gpsimd.dma_start(out=bo_sb, in_=bo_r)
            for i, (n0, nn) in enumerate(NCHUNKS):
                nc.gpsimd.dma_start(out=qnat_sb[0:nn, i, :], in_=qnat[n0 : n0 + nn, :])
            ones_bf = const.tile([1, P], BF16)
            nc.vector.memset(ones_bf, 1.0)

            # persistent big tensors
            KF_sb = persist.tile([P, 4, L], BF16)  # interleaved [k;kp] heads
            V33_sb = persist.tile([P, LCH, H, DH + 1], BF16)  # E*v + E col
            QF_sb = persist.tile([P, 4, N], BF16)
            outFT_sb = persist.tile([P, KC, N], BF16)
            recip_sb = persist.tile([1, H, N], BF16)

            # engine-balance trackers for the exp split
            bal = {"sc": SC_ATTN_FIXED, "dve": DVE_ATTN_FIXED}

            # ---- per-rep body (reps>1 only for benchmarking) ----
            rep_ctx = tc.For_i(0, reps, 1) if reps > 1 else None
            if rep_ctx is not None:
                rep_ctx.__enter__()
            if True:
                # denominator column = E row factors (refresh each rep)
                nc.vector.tensor_copy(out=V33_sb[:, :, :, DH], in_=E1_sb)
                if variant == "noattn":
                    nc.vector.memset(outFT_sb[0:1, 0, 0:1], 1.0)
                if variant == "noproj":
                    nc.vector.memset(KF_sb[0:1, 0, 0:1], 1.0)
                    nc.vector.memset(QF_sb[0:1, 0, 0:1], 1.0)
                    nc.vector.memset(V33_sb[0:1, 0, 0, 0:1], 1.0)

                # ---- projections, streamed over 8 l-groups of 512 ----
                for g in range(0 if variant == "noproj" else LG):
                    ksl = slice(g * LW, (g + 1) * LW)
                    kin = kin_pool.tile([P, KC, LW], BF16, name=f"kin{g}", tag="kin")
                    kpin = kin_pool.tile([P, KC, LW], BF16, name=f"kpin{g}", tag="kpin")
                    vin = kin_pool.tile([P, KC, LW], BF16, name=f"vin{g}", tag="vin")
                    if variant != "nodma":
                        nc.sync.dma_start(out=kin, in_=keyG[g])
                        nc.sync.dma_start(out=kpin, in_=kposG[g])
                        nc.sync.dma_start(out=vin, in_=valG[g])
                    else:
                        nc.vector.memset(kin[0:1, 0, 0:1], 1.0)
                        nc.vector.memset(kpin[0:1, 0, 0:1], 1.0)
                        nc.vector.memset(vin[0:1, 0, 0:1], 1.0)

                    # KF: psum = Ak.T @ key + Bk.T @ key_pos  (interleaved
                    # heads); 2 head-blocks per PSUM tile -> 1024-el evicts
                    for pbh in range(2):
                        kf_ps = wk.tile([P, GSZ, 512], F32, tag="work",
                                        name=f"kf_ps{g}_{pbh}")
                        for half in range(2):
                            pb = 2 * pbh + half
                            n_mm = 0
                            for w_sb, x_sb in ((Ak_sb, kin), (Bk_sb, kpin)):
                                for kc in range(KC):
                                    nc.tensor.matmul(
                                        kf_ps[:, half, :],
                                        (w_sb[:, kc, pb * P : (pb + 1) * P]),
                                        (x_sb[:, kc, :]),
                                        start=(n_mm == 0),
                                        stop=(n_mm == 3),
                                    )
                                    n_mm += 1
                        nc.scalar.activation(
                            out=KF_sb[:, 2 * pbh : 2 * pbh + 2, ksl],
                            in_=kf_ps[:, 0:2, :],
                            func=AF.Copy,
                        )

                    # V: natural layout; lhsT = valT slice (stationary), rhs =
                    # WvT; eviction fuses the E row-factor multiply
                    for sub2 in range(2):
                        v_ps = wk.tile([P, GSZ, 512], F32, tag="work",
                                       name=f"v_ps{g}_{sub2}")
                        for half in range(2):
                            sub = 2 * sub2 + half
                            for kc in range(KC):
                                nc.tensor.matmul(
                                    v_ps[:, half, 0:C],
                                    (vin[:, kc, sub * P : (sub + 1) * P]),
                                    (WvT_sb[:, kc, :]),
                                    start=(kc == 0),
                                    stop=(kc == KC - 1),
                                )
                        lidx = g * 4 + 2 * sub2
                        e_b = (
                            E1_sb[:, lidx : lidx + 2, :]
                            .rearrange("p s h -> p s h ()")
                            .broadcast_to([P, 2, H, DH])
                        )
                        nc.vector.tensor_mul(
                            V33_sb[:, lidx : lidx + 2, :, 0:DH],
                            v_ps[:, 0:2, 0:C].rearrange("p s (h d) -> p s h d", h=H),
                            e_b,
                        )

                    if g == 1:
                        # QF projection emitted here: keeps the PE free to
                        # start on KF immediately; ready long before QK
                        for pbh in range(2):
                            qf_ps = wk.tile([P, GSZ, 512], F32, tag="work",
                                            name=f"qf_ps{pbh}")
                            for half in range(2):
                                pb = 2 * pbh + half
                                n_mm = 0
                                for w_sb, x_sb in (
                                    (Aq_sb, qT_sb), (Bq_sb, qpT_sb), (Cq_sb, qsT_sb)
                                ):
                                    for kc in range(KC):
                                        nc.tensor.matmul(
                                            qf_ps[:, half, 0:N],
                                            (w_sb[:, kc, pb * P : (pb + 1) * P]),
                                            (x_sb[:, kc, :]),
                                            start=(n_mm == 0),
                                            stop=(n_mm == 5),
                                        )
                                        n_mm += 1
                            nc.scalar.activation(
                                out=QF_sb[:, 2 * pbh : 2 * pbh + 2, :],
                                in_=qf_ps[:, 0:2, 0:N],
                                func=AF.Copy,
                            )

                # ---- attention: jobs = (pb, parity, group of <=3 chunks) ---
                groups = [
                    list(range(s, min(s + GSZ, LCH))) for s in range(0, LCH, GSZ)
                ]
                jobs = []
                for pb in range(0 if variant == "noattn" else 4):
                    for gi, chunks in enumerate(groups):
                        for parity in range(2):
                            jobs.append((pb, parity, chunks, gi == len(groups) - 1))

                o2_tiles = {}
                at_tiles = {}

                def emit_qk(j):
                    pb, parity, chunks, _ = jobs[j]
                    at_t = wk.tile([P, GSZ, 512], F32, tag="work",
                                   name=f"at{j}")
                    at_tiles[j] = at_t
                    pq = 64 * parity
                    for i, lc in enumerate(chunks):
                        nc.tensor.matmul(
                            at_t[:, i, 0:N],
                            (KF_sb[pq : pq + 64, pb, lc * P : (lc + 1) * P]),
                            (QF_sb[pq : pq + 64, pb, :]),
                            start=True,
                            stop=True,
                        )

                def emit_exp_pv(j):
                    pb, parity, chunks, last = jobs[j]
                    h = 2 * pb + parity
                    key = (pb, parity)
                    if key not in o2_tiles:
                        o2_tiles[key] = o2p.tile(
                            [P, 512], F32, tag="o2", name=f"o2_{pb}_{parity}"
                        )
                    o2t = o2_tiles[key]
                    at_t = at_tiles.pop(j)
                    nch = len(chunks)
                    ptile = pt_pool.tile([P, GSZ, N], BF16, name=f"pt{j}", tag="pt")
                    if variant == "noexp":
                        use_sc = None
                    elif variant == "allsc":
                        use_sc = True
                    elif variant == "alldve":
                        use_sc = False
                    else:
                        use_sc = bal["sc"] + sc_exp_cost <= bal["dve"] + dve_exp_cost
                    if use_sc is None:
                        nc.vector.memset(ptile[0:1, 0, 0:1], 1.0)
                    elif use_sc:
                        bal["sc"] += sc_exp_cost * nch / GSZ
                        nc.scalar.activation(
                            out=ptile[:, 0:nch, :],
                            in_=at_t[:, 0:nch, 0:N],
                            func=AF.Exp,
                            scale=SCALE,
                        )
                    else:
                        bal["dve"] += dve_exp_cost * nch / GSZ
                        nc.vector.tensor_scalar(
                            out=ptile[:, 0:nch, :].bitcast(I16),
                            in0=at_t[:, 0:nch, 0:N],
                            scalar1=SCH_A,
                            scalar2=SCH_B,
                            op0=ALU.mult,
                            op1=ALU.add,
                        )
                    base = 64 * parity
                    for i, lc in enumerate(chunks):
                        nc.tensor.matmul(
                            o2t[base : base + DH + 1, 0:N],
                            (V33_sb[:, lc, h, :]),
                            (ptile[:, i, :]),
                            start=(chunks[0] == 0 and i == 0),
                            stop=(last and i == nch - 1),
                        )
                    if last:
                        emit_norm(pb, parity, o2t)

                def emit_norm(pb, parity, o2t):
                    h = 2 * pb + parity
                    base = 64 * parity
                    with nc.allow_low_precision(reason="bf16 recip feeds bcast"):
                        nc.vector.reciprocal(
                            out=recip_sb[0:1, h, :],
                            in_=o2t[base + DH : base + DH + 1, 0:N],
                        )
                    recipB = fin_pool.tile([DH, N], BF16, name=f"recipB{h}", tag="recipB")
                    if gp_bcast:
                        nc.gpsimd.partition_broadcast(recipB, recip_sb[0:1, h, :])
                    else:
                        rb_ps = wk.tile([P, GSZ, 512], F32, tag="work", name=f"rb_ps{h}")
                        nc.tensor.matmul(
                            rb_ps[0:DH, 0, 0:N],
                            (ones_bf[:, 0:DH]),
                            (recip_sb[0:1, h, :]),
                            start=True,
                            stop=True,
                        )
                        nc.scalar.activation(out=recipB, in_=rb_ps[0:DH, 0, 0:N], func=AF.Copy)
                    osl = outFT_sb[DH * (h % 4) : DH * (h % 4) + DH, h // 4, :]
                    nc.vector.tensor_mul(osl, o2t[base : base + DH, 0:N], recipB)

                LOOKAHEAD = 2
                if jobs:
                    for j in range(min(LOOKAHEAD, len(jobs))):
                        emit_qk(j)
                    for j in range(LOOKAHEAD, len(jobs)):
                        emit_qk(j)
                        emit_exp_pv(j - LOOKAHEAD)
                    for j in range(max(0, len(jobs) - LOOKAHEAD), len(jobs)):
                        emit_exp_pv(j)

                # ---- O-projection + residual ----
                for i, (n0, nn) in enumerate(NCHUNKS):
                    o3 = o2p.tile([P, 512], F32, tag="o2", name=f"o3_{i}")
                    for kc in range(KC):
                        nc.tensor.matmul(
                            o3[0:nn, 0:C],
                            (outFT_sb[:, kc, n0 : n0 + nn]),
                            (WoT_sb[:, kc, :]),
                            start=(kc == 0),
                            stop=False,
                        )
                    nc.tensor.matmul(
                        o3[0:nn, 0:C],
                        (ones_bf[:, 0:nn]),
                        (bo_sb),
                        start=False,
                        stop=True,
                    )
                    fin = fin_pool.tile([P, C], F32, name=f"fin{i}", tag="fin")
                    nc.vector.tensor_add(fin[0:nn, :], o3[0:nn, 0:C], qnat_sb[0:nn, i, :])
                    nc.sync.dma_start(out=out_d[n0 : n0 + nn, :], in_=fin[0:nn, :])

            if rep_ctx is not None:
                rep_ctx.__exit__(None, None, None)

    nc.compile()
    return nc


def _bf16(x):
    import ml_dtypes

    return np.asarray(x, dtype=np.float32).astype(ml_dtypes.bfloat16)


def prep_all_cores(inputs):
    """Host-side prep shared across cores: weights, E factors, folded biases."""
    f = np.float32
    W = {k: np.asarray(inputs["W" + k], dtype=f) for k in ("qc", "qp", "qs", "kc", "kp", "v", "o")}
    bias = {k: np.asarray(inputs["b" + k], dtype=f) for k in ("qc", "qp", "qs", "kc", "kp", "v", "o")}

    def interleave_w(Wa):
        """(256,256) weight -> (256, 512) with head h's 32 cols at 64h..64h+32."""
        out = np.zeros((C, 2 * C), dtype=f)
        WT = Wa.T  # (c, d)
        for h in range(H):
            out[:, 64 * h : 64 * h + DH] = WT[:, DH * h : DH * h + DH]
        return out

    def interleave_w_hi(Wa):
        out = np.zeros((C, 2 * C), dtype=f)
        WT = Wa.T
        for h in range(H):
            out[:, 64 * h + DH : 64 * h + 2 * DH] = WT[:, DH * h : DH * h + DH]
        return out

    shared = {
        "Ak": _bf16(interleave_w(W["kc"])),
        "Bk": _bf16(interleave_w(W["kp"]) + interleave_w_hi(W["kp"])),
        "Aq": _bf16(interleave_w(W["qc"])),
        "Bq": _bf16(interleave_w(W["qp"])),
        "Cq": _bf16(interleave_w_hi(W["qs"])),
        "WvT": _bf16(W["v"].T),
        "WoT": _bf16(W["o"].T),
        "bo_r": _bf16((bias["o"] + W["o"] @ bias["v"]).reshape(1, C)),
    }

    # E row factors: h_l[head] = bq_lo . (kc_u + kp_u)[head] + bqs . kp_u[head]
    key = np.asarray(inputs["key"], dtype=f)  # (B, L, C)
    key_pos = np.asarray(inputs["key_pos"], dtype=f)
    kc_u = np.einsum("blc,dc->bld", key, W["kc"])
    kp_u = np.einsum("blc,dc->bld", key_pos, W["kp"])
    klo = kc_u + kp_u
    bq_lo = (bias["qc"] + bias["qp"]).reshape(H, DH)
    bq_hi = bias["qs"].reshape(H, DH)
    hterm = (
        np.einsum("blhd,hd->blh", klo.reshape(B, L, H, DH), bq_lo)
        + np.einsum("blhd,hd->blh", kp_u.reshape(B, L, H, DH), bq_hi)
    )
    E = np.exp(SCALE * hterm)  # (B, L, H)
    shared["_E1"] = _bf16(E.reshape(B, LCH, P, H).transpose(0, 2, 1, 3))  # (B,P,LCH,H)
    return shared


def prep_core_inputs(inputs, b, shared=None):
    """Host-side prep: transpose activations to bf16 for core b."""
    if shared is None:
        shared = prep_all_cores(inputs)
    f = np.float32
    t = lambda x: _bf16(np.ascontiguousarray(np.asarray(x)[b].T))

    def g_major(x):
        # (C, L) transposed activation -> (LG, P, KC, LW) contiguous groups
        a = t(x)  # (C, L) bf16
        return np.ascontiguousarray(
            a.reshape(KC, P, LG, LW).transpose(2, 1, 0, 3)
        )

    m = {
        "qT": t(inputs["query"]),
        "qpT": t(inputs["query_pos"]),
        "qsT": t(inputs["query_sine_embed"]),
        "keyG": g_major(inputs["key"]),
        "kposG": g_major(inputs["key_pos"]),
        "valG": g_major(inputs["value"]),
        "qnat": np.ascontiguousarray(np.asarray(inputs["query"])[b], dtype=f),
        "E1": np.ascontiguousarray(shared["_E1"][b]),
    }
    for k in ("Ak", "Bk", "Aq", "Bq", "Cq", "WvT", "WoT", "bo_r"):
        m[k] = shared[k]
    return m


_NC_CACHE = {}


def get_nc():
    if "nc" not in _NC_CACHE:
        _NC_CACHE["nc"] = build_nc()
    return _NC_CACHE["nc"]


def kernel(**inputs):
    nc = get_nc()
    shared = prep_all_cores(inputs)
    in_maps = [prep_core_inputs(inputs, b, shared) for b in range(B)]
    res = run_bass_kernel_spmd(nc, in_maps, core_ids=list(range(B)))
    return np.stack([res.results[b]["out"] for b in range(B)]).astype(np.float32)


# revision 21
# speedup vs baseline: 1.2332x; 1.2332x over previous
"""Trainium2 Bass kernel for ConditionalCrossAttention (DAB-DETR style).

Reference computation (per batch b):
    qc = query @ Wqc.T + bqc ; qp = query_pos @ Wqp.T + bqp ; qs = qsine @ Wqs.T + bqs
    kc = key @ Wkc.T + bkc   ; kp = key_pos @ Wkp.T + bkp   ; v = value @ Wv.T + bv
    q_full = concat_heads(qc+qp, qs)   # (N, H, 64)
    k_full = concat_heads(kc+kp, kp)   # (L, H, 64)
    attn = softmax(q_full . k_full / 8) ; out = attn @ v_heads
    out = out @ Wo.T + bo ; return query + out

Sharding: data-parallel over batch B=8 across the 8 NeuronCores; each core
computes one batch element end to end (no collectives).

v3 design (all-bf16 matmuls, bias algebra folded away, drain-op minimized):
 - Softmax over l is invariant to per-n and constant logit shifts, so the
   K-side biases and bq.bk cancel entirely.  The only surviving bias term is
   h_l = bq_full . k_features(l), handled as a multiplicative row factor
   E = exp(h/8) on V (and on the denominator column), computed on the HOST
   from the inputs and shipped as a small (L, H) bf16 tensor.
 - V bias: attn weights sum to 1, so +bv passes through PV; folded host-side
   into the output projection bias bo' = bo + Wo bv.
 - Projections: interleaved [k;kp] x H "KF" (512, L) and "QF" (512, N) in
   bf16 out of PSUM; KF evictions are consolidated to 1024-element ScalarE
   copies (2 head-blocks per PSUM tile), DVE evicts V with the E row-factor
   multiply fused (broadcast in1).
 - Attention jobs = (head-pair block, parity, group of <=3 l-chunks); one
   [128,3,512] PSUM tile per job gives 900-element exp ops (HW per-op
   overhead dominates, so fewer/bigger drain ops win).  QK matmuls contract
   K=64 at base partitions 0/64 (disjoint PE row groups); PV accumulates
   into per-parity PSUM banks at partitions 0/64 (disjoint col groups).
   QK for job j+1 is emitted before exp/PV of job j so the PE never stalls
   behind the exponentials.  exp is alternated between ScalarE (LUT exp)
   and DVE (Schraudolph bit-trick exp via int16 affine + bf16 bitcast).
 - Normalization: reciprocal of the E-weighted denominator row, broadcast
   over 32 partitions on GpSimd, multiply on VectorE.
 - O-projection in bf16, residual add with fp32 query, DMA out fp32.
"""

import sys

for _p in ("/opt/trn_rl_repo",):
    if _p not in sys.path:
        sys.path.insert(0, _p)

import numpy as np

import concourse.bass as bass
import concourse.mybir as mybir
import concourse.tile as tile
from concourse import bacc
from concourse.bass_utils import run_bass_kernel_spmd

B, N, L, C, H = 8, 300, 4096, 256, 8
DH = C // H  # 32
P = 128
KC = C // P  # 2 contraction chunks of 128
LG = 8  # l groups of 512 for projection streaming
LW = L // LG  # 512
LCH = L // P  # 32 l-chunks of 128 for attention
NCHUNKS = [(0, 128), (128, 128), (256, 44)]  # n tiling of 300
GSZ = 2  # l-chunks per attention job

F32 = mybir.dt.float32
BF16 = mybir.dt.bfloat16
I16 = mybir.dt.int16
AF = mybir.ActivationFunctionType
ALU = mybir.AluOpType

SCALE = 0.125
# Schraudolph exp constants for bf16 bit pattern: bits = round(x*As + Bs)
SCH_A = SCALE * 1.4426950408889634 * 128.0
SCH_B = 16256.0 - 8.5

# engine-balance weights for the exp split (relative, HW-calibrated)
COST_SC_EXP = 1500.0
COST_DVE_EXP = 1500.0
SC_ATTN_FIXED = 0.0
DVE_ATTN_FIXED = 13.0e3  # recip + norm-mul + residual


def build_nc(reps=1, variant="full", sc_exp_cost=COST_SC_EXP, dve_exp_cost=COST_DVE_EXP,
             gp_bcast=True):
    """variant: full | allsc | alldve | noattn | noproj | noexp | nodma
    (ablations for HW timing attribution; only "full" is numerically correct).
    sc_exp_cost/dve_exp_cost: relative weights for the exp engine balance.
    gp_bcast: broadcast the reciprocal row on GpSimd instead of PE+ScalarE."""
    nc = bacc.Bacc(trn_type="TRN2", debug=False, enable_partition_id=False)

    def din(name, shape, dt=BF16):
        return nc.dram_tensor(name, list(shape), dt, kind="ExternalInput").ap()

    # transposed activations (channels, tokens), bf16
    qT = din("qT", (C, N))
    qpT = din("qpT", (C, N))
    qsT = din("qsT", (C, N))
    # group-major layout [g][p][k][x]: each DMA group is one contiguous
    # 256KB block with 2KB per-partition lines
    keyG = din("keyG", (LG, P, KC, LW))
    kposG = din("kposG", (LG, P, KC, LW))
    valG = din("valG", (LG, P, KC, LW))
    qnat = din("qnat", (N, C), F32)  # natural query for the residual
    # host-prepped weights (interleaved, unbiased)
    Ak = din("Ak", (C, 512))
    Bk = din("Bk", (C, 512))
    Aq = din("Aq", (C, 512))
    Bq = din("Bq", (C, 512))
    Cq = din("Cq", (C, 512))
    WvT = din("WvT", (C, C))
    WoT = din("WoT", (C, C))
    E1 = din("E1", (P, LCH, H))  # exp(h/8) row factors, l = c*128+p
    bo_r = din("bo_r", (1, C))  # bo + Wo bv
    out_d = nc.dram_tensor("out", [N, C], F32, kind="ExternalOutput").ap()

    # (c, x) dram tensors viewed as (partition, chunk, x)
    def pkx(ap):
        return ap.rearrange("(k p) x -> p k x", p=P)

    with tile.TileContext(nc) as tc:
        with (
            tc.tile_pool(name="const", bufs=1) as const,
            tc.tile_pool(name="persist", bufs=1) as persist,
            tc.tile_pool(name="kin", bufs=4) as kin_pool,
            tc.tile_pool(name="pt", bufs=4) as pt_pool,
            tc.tile_pool(name="fin", bufs=2) as fin_pool,
            tc.tile_pool(name="wk", bufs=3, space="PSUM") as wk,
            tc.tile_pool(name="o2p", bufs=2, space="PSUM") as o2p,
        ):
            # ---- constants / weights (one-time DMAs via SWDGE queue) ----
            Ak_sb = const.tile([P, KC, 512], BF16)
            Bk_sb = const.tile([P, KC, 512], BF16)
            Aq_sb = const.tile([P, KC, 512], BF16)
            Bq_sb = const.tile([P, KC, 512], BF16)
            Cq_sb = const.tile([P, KC, 512], BF16)
            WvT_sb = const.tile([P, KC, C], BF16)
            WoT_sb = const.tile([P, KC, C], BF16)
            E1_sb = const.tile([P, LCH, H], BF16)
            bo_sb = const.tile([1, C], BF16)
            qnat_sb = const.tile([P, 3, C], F32)
            qT_sb = persist.tile([P, KC, N], BF16)
            qpT_sb = persist.tile([P, KC, N], BF16)
            qsT_sb = persist.tile([P, KC, N], BF16)
            nc.gpsimd.dma_start(out=Ak_sb, in_=pkx(Ak))
            nc.gpsimd.dma_start(out=Bk_sb, in_=pkx(Bk))
            nc.gpsimd.dma_start(out=WvT_sb, in_=pkx(WvT))
            nc.gpsimd.dma_start(out=qT_sb, in_=pkx(qT))
            nc.gpsimd.dma_start(out=qpT_sb, in_=pkx(qpT))
            nc.gpsimd.dma_start(out=qsT_sb, in_=pkx(qsT))
            nc.gpsimd.dma_start(out=E1_sb, in_=E1)
            for dst, src in ((Aq_sb, Aq), (Bq_sb, Bq), (Cq_sb, Cq), (WoT_sb, WoT)):
                nc.gpsimd.dma_start(out=dst, in_=pkx(src))
            nc.gpsimd.dma_start(out=bo_sb, in_=bo_r)
            for i, (n0, nn) in enumerate(NCHUNKS):
                nc.gpsimd.dma_start(out=qnat_sb[0:nn, i, :], in_=qnat[n0 : n0 + nn, :])
            ones_bf = const.tile([1, P], BF16)
            nc.vector.memset(ones_bf, 1.0)

            # persistent big tensors
            KF_sb = persist.tile([P, 4, L], BF16)  # interleaved [k;kp] heads
            V33_sb = persist.tile([P, LCH, H, DH + 1], BF16)  # E*v + E col
            QF_sb = persist.tile([P, 4, N], BF16)
            outFT_sb = persist.tile([P, KC, N], BF16)
            recip_sb = persist.tile([1, H, N], BF16)

            # engine-balance trackers for the exp split
            bal = {"sc": SC_ATTN_FIXED, "dve": DVE_ATTN_FIXED}

            # ---- per-rep body (reps>1 only for benchmarking) ----
            rep_ctx = tc.For_i(0, reps, 1) if reps > 1 else None
            if rep_ctx is not None:
                rep_ctx.__enter__()
            if True:
                # denominator column = E row factors (refresh each rep)
                nc.vector.tensor_copy(out=V33_sb[:, :, :, DH], in_=E1_sb)
                if variant == "noattn":
                    nc.vector.memset(outFT_sb[0:1, 0, 0:1], 1.0)
                if variant == "noproj":
                    nc.vector.memset(KF_sb[0:1, 0, 0:1], 1.0)
                    nc.vector.memset(QF_sb[0:1, 0, 0:1], 1.0)
                    nc.vector.memset(V33_sb[0:1, 0, 0, 0:1], 1.0)

                # ---- attention pipeline helpers (jobs pushed both from
                # inside the projection loop for head-block 0 and after it) --
                o2_tiles = {}
                pipe = []  # (job, at_tile) awaiting exp+PV
                cnt = {"n": 0}
                LOOKAHEAD = 2

                def emit_qk(job):
                    pb, parity, chunks, _ = job
                    cnt["n"] += 1
                    at_t = wk.tile([P, GSZ, 512], F32, tag="work",
                                   name=f"at{cnt['n']}")
                    pq = 64 * parity
                    for i, lc in enumerate(chunks):
                        nc.tensor.matmul(
                            at_t[:, i, 0:N],
                            (KF_sb[pq : pq + 64, pb, lc * P : (lc + 1) * P]),
                            (QF_sb[pq : pq + 64, pb, :]),
                            start=True,
                            stop=True,
                        )
                    return at_t

                def emit_exp_pv(job, at_t):
                    pb, parity, chunks, last = job
                    h = 2 * pb + parity
                    key = (pb, parity)
                    if key not in o2_tiles:
                        o2_tiles[key] = o2p.tile(
                            [P, 512], F32, tag="o2", name=f"o2_{pb}_{parity}"
                        )
                    o2t = o2_tiles[key]
                    nch = len(chunks)
                    ptile = pt_pool.tile([P, GSZ, N], BF16,
                                         name=f"pt{cnt['n']}", tag="pt")
                    if variant == "noexp":
                        use_sc = None
                    elif variant == "allsc":
                        use_sc = True
                    elif variant == "alldve":
                        use_sc = False
                    else:
                        use_sc = bal["sc"] + sc_exp_cost <= bal["dve"] + dve_exp_cost
                    if use_sc is None:
                        nc.vector.memset(ptile[0:1, 0, 0:1], 1.0)
                    elif use_sc:
                        bal["sc"] += sc_exp_cost * nch / GSZ
                        nc.scalar.activation(
                            out=ptile[:, 0:nch, :],
                            in_=at_t[:, 0:nch, 0:N],
                            func=AF.Exp,
                            scale=SCALE,
                        )
                    else:
                        bal["dve"] += dve_exp_cost * nch / GSZ
                        nc.vector.tensor_scalar(
                            out=ptile[:, 0:nch, :].bitcast(I16),
                            in0=at_t[:, 0:nch, 0:N],
                            scalar1=SCH_A,
                            scalar2=SCH_B,
                            op0=ALU.mult,
                            op1=ALU.add,
                        )
                    base = 64 * parity
                    for i, lc in enumerate(chunks):
                        nc.tensor.matmul(
                            o2t[base : base + DH + 1, 0:N],
                            (V33_sb[:, lc, h, :]),
                            (ptile[:, i, :]),
                            start=(chunks[0] == 0 and i == 0),
                            stop=(last and i == nch - 1),
                        )
                    if last:
                        emit_norm(pb, parity, o2t)

                def emit_norm(pb, parity, o2t):
                    h = 2 * pb + parity
                    base = 64 * parity
                    with nc.allow_low_precision(reason="bf16 recip feeds bcast"):
                        nc.vector.reciprocal(
                            out=recip_sb[0:1, h, :],
                            in_=o2t[base + DH : base + DH + 1, 0:N],
                        )
                    recipB = fin_pool.tile([DH, N], BF16, name=f"recipB{h}", tag="recipB")
                    if gp_bcast:
                        nc.gpsimd.partition_broadcast(recipB, recip_sb[0:1, h, :])
                    else:
                        rb_ps = wk.tile([P, GSZ, 512], F32, tag="work", name=f"rb_ps{h}")
                        nc.tensor.matmul(
                            rb_ps[0:DH, 0, 0:N],
                            (ones_bf[:, 0:DH]),
                            (recip_sb[0:1, h, :]),
                            start=True,
                            stop=True,
                        )
                        nc.scalar.activation(out=recipB, in_=rb_ps[0:DH, 0, 0:N], func=AF.Copy)
                    osl = outFT_sb[DH * (h % 4) : DH * (h % 4) + DH, h // 4, :]
                    nc.vector.tensor_mul(osl, o2t[base : base + DH, 0:N], recipB)

                def push_job(job):
                    at_t = emit_qk(job)
                    pipe.append((job, at_t))
                    if len(pipe) > LOOKAHEAD:
                        emit_exp_pv(*pipe.pop(0))

                def flush_jobs():
                    while pipe:
                        emit_exp_pv(*pipe.pop(0))

                # ---- projections, streamed over 8 l-groups of 512 ----
                for g in range(0 if variant == "noproj" else LG):
                    ksl = slice(g * LW, (g + 1) * LW)
                    kin = kin_pool.tile([P, KC, LW], BF16, name=f"kin{g}", tag="kin")
                    kpin = kin_pool.tile([P, KC, LW], BF16, name=f"kpin{g}", tag="kpin")
                    vin = kin_pool.tile([P, KC, LW], BF16, name=f"vin{g}", tag="vin")
                    if variant != "nodma":
                        nc.sync.dma_start(out=kin, in_=keyG[g])
                        nc.sync.dma_start(out=kpin, in_=kposG[g])
                        nc.sync.dma_start(out=vin, in_=valG[g])
                    else:
                        nc.vector.memset(kin[0:1, 0, 0:1], 1.0)
                        nc.vector.memset(kpin[0:1, 0, 0:1], 1.0)
                        nc.vector.memset(vin[0:1, 0, 0:1], 1.0)

                    # KF: psum = Ak.T @ key + Bk.T @ key_pos  (interleaved
                    # heads); 2 head-blocks per PSUM tile -> 1024-el evicts
                    for pbh in range(2):
                        kf_ps = wk.tile([P, GSZ, 512], F32, tag="work",
                                        name=f"kf_ps{g}_{pbh}")
                        for half in range(2):
                            pb = 2 * pbh + half
                            n_mm = 0
                            for w_sb, x_sb in ((Ak_sb, kin), (Bk_sb, kpin)):
                                for kc in range(KC):
                                    nc.tensor.matmul(
                                        kf_ps[:, half, :],
                                        (w_sb[:, kc, pb * P : (pb + 1) * P]),
                                        (x_sb[:, kc, :]),
                                        start=(n_mm == 0),
                                        stop=(n_mm == 3),
                                    )
                                    n_mm += 1
                        nc.scalar.activation(
                            out=KF_sb[:, 2 * pbh : 2 * pbh + 2, ksl],
                            in_=kf_ps[:, 0:2, :],
                            func=AF.Copy,
                        )

                    # V: natural layout; lhsT = valT slice (stationary), rhs =
                    # WvT; eviction fuses the E row-factor multiply
                    for sub2 in range(2):
                        v_ps = wk.tile([P, GSZ, 512], F32, tag="work",
                                       name=f"v_ps{g}_{sub2}")
                        for half in range(2):
                            sub = 2 * sub2 + half
                            for kc in range(KC):
                                nc.tensor.matmul(
                                    v_ps[:, half, 0:C],
                                    (vin[:, kc, sub * P : (sub + 1) * P]),
                                    (WvT_sb[:, kc, :]),
                                    start=(kc == 0),
                                    stop=(kc == KC - 1),
                                )
                        for half in range(2):
                            lidx = g * 4 + 2 * sub2 + half
                            e_b = (
                                E1_sb[:, lidx, :]
                                .rearrange("p h -> p h ()")
                                .broadcast_to([P, H, DH])
                            )
                            nc.vector.tensor_mul(
                                V33_sb[:, lidx, :, 0:DH],
                                v_ps[:, half, 0:C].rearrange("p (h d) -> p h d", h=H),
                                e_b,
                            )

                    if g == 0:
                        # QF projection emitted here: keeps the PE free to
                        # start on KF immediately; ready long before QK
                        for pbh in range(2):
                            qf_ps = wk.tile([P, GSZ, 512], F32, tag="work",
                                            name=f"qf_ps{pbh}")
                            for half in range(2):
                                pb = 2 * pbh + half
                                n_mm = 0
                                for w_sb, x_sb in (
                                    (Aq_sb, qT_sb), (Bq_sb, qpT_sb), (Cq_sb, qsT_sb)
                                ):
                                    for kc in range(KC):
                                        nc.tensor.matmul(
                                            qf_ps[:, half, 0:N],
                                            (w_sb[:, kc, pb * P : (pb + 1) * P]),
                                            (x_sb[:, kc, :]),
                                            start=(n_mm == 0),
                                            stop=(n_mm == 5),
                                        )
                                        n_mm += 1
                            nc.scalar.activation(
                                out=QF_sb[:, 2 * pbh : 2 * pbh + 2, :],
                                in_=qf_ps[:, 0:2, 0:N],
                                func=AF.Copy,
                            )

                    if variant not in ("noattn", "noproj"):
                        # head-block 0 attention streams behind the projections
                        for cpair in range(2):
                            chunks = [4 * g + 2 * cpair, 4 * g + 2 * cpair + 1]
                            for parity in range(2):
                                push_job((0, parity, chunks, chunks[-1] == LCH - 1))

                groups2 = [
                    list(range(s, s + GSZ)) for s in range(0, LCH, GSZ)
                ]
                pb_rest = (
                    []
                    if variant == "noattn"
                    else (range(4) if variant == "noproj" else range(1, 4))
                )
                for pb in pb_rest:
                    for gi, chunks in enumerate(groups2):
                        for parity in range(2):
                            push_job((pb, parity, chunks, gi == len(groups2) - 1))
                flush_jobs()

                # ---- O-projection + residual ----
                for i, (n0, nn) in enumerate(NCHUNKS):
                    o3 = o2p.tile([P, 512], F32, tag="o2", name=f"o3_{i}")
                    for kc in range(KC):
                        nc.tensor.matmul(
                            o3[0:nn, 0:C],
                            (outFT_sb[:, kc, n0 : n0 + nn]),
                            (WoT_sb[:, kc, :]),
                            start=(kc == 0),
                            stop=False,
                        )
                    nc.tensor.matmul(
                        o3[0:nn, 0:C],
                        (ones_bf[:, 0:nn]),
                        (bo_sb),
                        start=False,
                        stop=True,
                    )
                    fin = fin_pool.tile([P, C], F32, name=f"fin{i}", tag="fin")
                    nc.vector.tensor_add(fin[0:nn, :], o3[0:nn, 0:C], qnat_sb[0:nn, i, :])
                    nc.sync.dma_start(out=out_d[n0 : n0 + nn, :], in_=fin[0:nn, :])

            if rep_ctx is not None:
                rep_ctx.__exit__(None, None, None)

    nc.compile()
    return nc


def _bf16(x):
    import ml_dtypes

    return np.asarray(x, dtype=np.float32).astype(ml_dtypes.bfloat16)


def prep_all_cores(inputs):
    """Host-side prep shared across cores: weights, E factors, folded biases."""
    f = np.float32
    W = {k: np.asarray(inputs["W" + k], dtype=f) for k in ("qc", "qp", "qs", "kc", "kp", "v", "o")}
    bias = {k: np.asarray(inputs["b" + k], dtype=f) for k in ("qc", "qp", "qs", "kc", "kp", "v", "o")}

    def interleave_w(Wa):
        """(256,256) weight -> (256, 512) with head h's 32 cols at 64h..64h+32."""
        out = np.zeros((C, 2 * C), dtype=f)
        WT = Wa.T  # (c, d)
        for h in range(H):
            out[:, 64 * h : 64 * h + DH] = WT[:, DH * h : DH * h + DH]
        return out

    def interleave_w_hi(Wa):
        out = np.zeros((C, 2 * C), dtype=f)
        WT = Wa.T
        for h in range(H):
            out[:, 64 * h + DH : 64 * h + 2 * DH] = WT[:, DH * h : DH * h + DH]
        return out

    shared = {
        "Ak": _bf16(interleave_w(W["kc"])),
        "Bk": _bf16(interleave_w(W["kp"]) + interleave_w_hi(W["kp"])),
        "Aq": _bf16(interleave_w(W["qc"])),
        "Bq": _bf16(interleave_w(W["qp"])),
        "Cq": _bf16(interleave_w_hi(W["qs"])),
        "WvT": _bf16(W["v"].T),
        "WoT": _bf16(W["o"].T),
        "bo_r": _bf16((bias["o"] + W["o"] @ bias["v"]).reshape(1, C)),
    }

    # E row factors: h_l[head] = bq_lo . (kc_u + kp_u)[head] + bqs . kp_u[head]
    key = np.asarray(inputs["key"], dtype=f)  # (B, L, C)
    key_pos = np.asarray(inputs["key_pos"], dtype=f)
    kc_u = np.einsum("blc,dc->bld", key, W["kc"])
    kp_u = np.einsum("blc,dc->bld", key_pos, W["kp"])
    klo = kc_u + kp_u
    bq_lo = (bias["qc"] + bias["qp"]).reshape(H, DH)
    bq_hi = bias["qs"].reshape(H, DH)
    hterm = (
        np.einsum("blhd,hd->blh", klo.reshape(B, L, H, DH), bq_lo)
        + np.einsum("blhd,hd->blh", kp_u.reshape(B, L, H, DH), bq_hi)
    )
    E = np.exp(SCALE * hterm)  # (B, L, H)
    shared["_E1"] = _bf16(E.reshape(B, LCH, P, H).transpose(0, 2, 1, 3))  # (B,P,LCH,H)
    return shared


def prep_core_inputs(inputs, b, shared=None):
    """Host-side prep: transpose activations to bf16 for core b."""
    if shared is None:
        shared = prep_all_cores(inputs)
    f = np.float32
    t = lambda x: _bf16(np.ascontiguousarray(np.asarray(x)[b].T))

    def g_major(x):
        # (C, L) transposed activation -> (LG, P, KC, LW) contiguous groups
        a = t(x)  # (C, L) bf16
        return np.ascontiguousarray(
            a.reshape(KC, P, LG, LW).transpose(2, 1, 0, 3)
        )

    m = {
        "qT": t(inputs["query"]),
        "qpT": t(inputs["query_pos"]),
        "qsT": t(inputs["query_sine_embed"]),
        "keyG": g_major(inputs["key"]),
        "kposG": g_major(inputs["key_pos"]),
        "valG": g_major(inputs["value"]),
        "qnat": np.ascontiguousarray(np.asarray(inputs["query"])[b], dtype=f),
        "E1": np.ascontiguousarray(shared["_E1"][b]),
    }
    for k in ("Ak", "Bk", "Aq", "Bq", "Cq", "WvT", "WoT", "bo_r"):
        m[k] = shared[k]
    return m


_NC_CACHE = {}


def get_nc():
    if "nc" not in _NC_CACHE:
        _NC_CACHE["nc"] = build_nc()
    return _NC_CACHE["nc"]


def kernel(**inputs):
    nc = get_nc()
    shared = prep_all_cores(inputs)
    in_maps = [prep_core_inputs(inputs, b, shared) for b in range(B)]
    res = run_bass_kernel_spmd(nc, in_maps, core_ids=list(range(B)))
    return np.stack([res.results[b]["out"] for b in range(B)]).astype(np.float32)
